# revision 7
# baseline (speedup 1.0000x reference)
"""Bass/Trainium2 kernel for a single-head causal decoder attention head.

Reference computation (fp32):
    k = x @ Wk; q = x @ Wq; v = x @ Wv            # [B,T,H]
    att = softmax(causal(q k^T / sqrt(H)))        # [B,T,T]
    out = att @ v                                 # [B,T,H]
with B=4, T=4096, C=1024, H=128.

Sharding: 8 cores = 4 batches x 2 query-interleave lanes (j in {0,1}).
Core (b, j) handles q-groups [(2i+j)*512, (2i+j+1)*512) for i in 0..3 and
runs a *uniform* kv-span schedule {1024, 2048, 3072, 4096} for groups
0..3, so all 8 cores execute the same instruction stream (SPMD, one NEFF)
while per-core DRAM data (x^T slices, q-column gather, mask stack) makes
the math come out right.  Causality beyond each group's true span is
enforced by additive -30000 masks on the last 8 kv chunks of each group.

Dataflow (per core, transposed land so no on-chip transposes are needed):
    KT [H, T]   = Wk^T x^T        (8 c-chunk matmuls per 512 kv cols)
    QT [H, 2048]= Wq^T xq^T
    V  [kv, H]  (32 blocks)       (lhsT = x^T chunk, rhs = Wv chunk)
    per q-group, per kv chunk c:
        S^T  = KT_c^T QT_g                 (PSUM [128kv, 512q])
        S^T += mask (last 8 chunks, DVE)
        P^T  = exp(S^T / sqrt(H))          (ACT, bf16 -> SBUF)
        outT += V_c^T P^T                  (PSUM [128H, 512q], accumulated)
        sums += ones^T P^T                 (PSUM [1, 512q], accumulated)
    outT / sums -> DRAM  (reciprocal + partition-broadcast + multiply)
"""

import sys

sys.path.insert(0, "/opt/trn_rl_repo")

import numpy as np
import ml_dtypes

import concourse.bass as bass
import concourse.mybir as mybir
import concourse.tile as tile
from concourse import bacc
from concourse.alu_op_type import AluOpType
from concourse.bass_utils import run_bass_kernel_spmd

B, T, C, H = 4, 4096, 1024, 128
NCORES = 8
QG = 512                      # q-group width
NG = 4                        # q-groups per core
SPANS = [1024, 2048, 3072, 4096]  # uniform kv span per group index
CB = C // 128                 # 8 contraction chunks
TGRP = T // QG                # 8 kv col-groups for projections
SCALE = float(H) ** -0.5
MASKVAL = -30000.0

BF16 = mybir.dt.bfloat16
F32 = mybir.dt.float32
NPBF16 = ml_dtypes.bfloat16


def _build_program():
    nc = bacc.Bacc("TRN2", target_bir_lowering=False, debug=False)

    xt = nc.dram_tensor("xt", [C, T], BF16, kind="ExternalInput").ap()
    xtq = nc.dram_tensor("xtq", [C, NG * QG], BF16, kind="ExternalInput").ap()
    wk = nc.dram_tensor("wk", [C, H], BF16, kind="ExternalInput").ap()
    wq = nc.dram_tensor("wq", [C, H], BF16, kind="ExternalInput").ap()
    wv = nc.dram_tensor("wv", [C, H], BF16, kind="ExternalInput").ap()
    msk = nc.dram_tensor("msk", [8, 128, QG], F32, kind="ExternalInput").ap()
    outT = nc.dram_tensor("outT", [H, NG * QG], F32, kind="ExternalOutput").ap()

    with tile.TileContext(nc) as tc:
        with (
            tc.tile_pool(name="const", bufs=1) as constp,
            tc.tile_pool(name="kvq", bufs=1) as kvqp,
            tc.tile_pool(name="xin", bufs=2) as xinp,
            tc.tile_pool(name="attb", bufs=3) as attp,
            tc.tile_pool(name="epi", bufs=2) as epip,
        ):
            # --- persistent SBUF tensors ---
            wks = constp.tile([128, CB * H], BF16, tag="wks")
            wqs = constp.tile([128, CB * H], BF16, tag="wqs")
            wvs = constp.tile([128, CB * H], BF16, tag="wvs")
            for c in range(CB):
                nc.sync.dma_start(wks[:, c * H:(c + 1) * H], wk[c * 128:(c + 1) * 128, :])
                nc.sync.dma_start(wqs[:, c * H:(c + 1) * H], wq[c * 128:(c + 1) * 128, :])
                nc.sync.dma_start(wvs[:, c * H:(c + 1) * H], wv[c * 128:(c + 1) * 128, :])
            masks = constp.tile([128, 8 * QG], F32, tag="masks")
            for m in range(8):
                nc.sync.dma_start(masks[:, m * QG:(m + 1) * QG], msk[m])
            ones = constp.tile([128, 1], BF16, tag="ones")
            nc.vector.memset(ones, 1.0)

            KT = kvqp.tile([128, T], BF16, tag="KT")
            VV = kvqp.tile([128, (T // 128) * H], BF16, tag="VV")
            QT = kvqp.tile([128, NG * QG], BF16, tag="QT")

            # --- phase 1: projections ---
            with tc.tile_pool(name="pp", bufs=2, space="PSUM") as ppool:
                for tg in range(TGRP):
                    xg = xinp.tile([128, CB * QG], BF16, tag="xg", bufs=TGRP)
                    for c in range(CB):
                        nc.sync.dma_start(
                            xg[:, c * QG:(c + 1) * QG],
                            xt[c * 128:(c + 1) * 128, tg * QG:(tg + 1) * QG],
                        )
                    kps = ppool.tile([128, QG], F32, tag="kps")
                    for c in range(CB):
                        nc.tensor.matmul(
                            kps,
                            lhsT=wks[:, c * H:(c + 1) * H],
                            rhs=xg[:, c * QG:(c + 1) * QG],
                            start=(c == 0),
                            stop=(c == CB - 1),
                        )
                    nc.any.tensor_copy(KT[:, tg * QG:(tg + 1) * QG], kps)
                    for tb in range(QG // 128):
                        t = tg * (QG // 128) + tb
                        vps = ppool.tile([128, H], F32, tag="vps")
                        for c in range(CB):
                            nc.tensor.matmul(
                                vps,
                                lhsT=xg[:, c * QG + tb * 128: c * QG + tb * 128 + 128],
                                rhs=wvs[:, c * H:(c + 1) * H],
                                start=(c == 0),
                                stop=(c == CB - 1),
                            )
                        nc.any.tensor_copy(VV[:, t * H:(t + 1) * H], vps)
                for i in range(NG):
                    xq = xinp.tile([128, CB * QG], BF16, tag="xq", bufs=NG)
                    for c in range(CB):
                        nc.sync.dma_start(
                            xq[:, c * QG:(c + 1) * QG],
                            xtq[c * 128:(c + 1) * 128, i * QG:(i + 1) * QG],
                        )
                    qps = ppool.tile([128, QG], F32, tag="qps")
                    for c in range(CB):
                        nc.tensor.matmul(
                            qps,
                            lhsT=wqs[:, c * H:(c + 1) * H],
                            rhs=xq[:, c * QG:(c + 1) * QG],
                            start=(c == 0),
                            stop=(c == CB - 1),
                        )
                    nc.any.tensor_copy(QT[:, i * QG:(i + 1) * QG], qps)

            # --- phase 2: attention ---
            with tc.tile_pool(name="ap", bufs=2, space="PSUM") as apool:
                for i in range(NG):
                    span = SPANS[i]
                    nchunks = span // 128
                    otps = apool.tile([128, QG], F32, tag="otps")
                    smps = apool.tile([1, QG], F32, tag="smps")
                    qg = QT[:, i * QG:(i + 1) * QG]
                    for c in range(nchunks):
                        sps = apool.tile([128, QG], F32, tag="sps")
                        nc.tensor.matmul(
                            sps,
                            lhsT=KT[:, c * 128:(c + 1) * 128],
                            rhs=qg,
                            start=True,
                            stop=True,
                        )
                        m = c - (nchunks - 8)
                        if m >= 0:
                            nc.vector.tensor_tensor(
                                sps, sps, masks[:, m * QG:(m + 1) * QG],
                                op=AluOpType.add,
                            )
                        pt = attp.tile([128, QG], BF16, tag="pt")
                        nc.scalar.activation(
                            pt, sps, mybir.ActivationFunctionType.Exp, scale=SCALE
                        )
                        nc.tensor.matmul(
                            otps,
                            lhsT=VV[:, c * H:(c + 1) * H],
                            rhs=pt,
                            start=(c == 0),
                            stop=(c == nchunks - 1),
                        )
                        nc.tensor.matmul(
                            smps,
                            lhsT=ones,
                            rhs=pt,
                            start=(c == 0),
                            stop=(c == nchunks - 1),
                        )
                    rec = epip.tile([1, QG], F32, tag="rec")
                    nc.vector.reciprocal(rec, smps)
                    rb = epip.tile([128, QG], F32, tag="rb")
                    nc.gpsimd.partition_broadcast(rb, rec)
                    ot = epip.tile([128, QG], F32, tag="ot")
                    nc.vector.tensor_tensor(ot, otps, rb, op=AluOpType.mult)
                    nc.sync.dma_start(outT[:, i * QG:(i + 1) * QG], ot)

    if not nc.is_finalized():
        nc.finalize()
    return nc


_NC_CACHE = None


def _get_program():
    global _NC_CACHE
    if _NC_CACHE is None:
        _NC_CACHE = _build_program()
    return _NC_CACHE


def _make_masks(j: int) -> np.ndarray:
    """Mask stack [8, 128, QG] for lane j (f32, 0 or MASKVAL).

    Slot s applies to kv chunk at offset K0 = g - (1024 - j*512) + 128*s
    relative ... concretely: for lane j, the last 8 chunks of each group's
    span get slots 0..7; masked iff global kv > global q, i.e.
    128*(s - 4 + (1 - j) * 4 ... reduces to: kv_i + 128*s - (4 - 4*j)*128 > q_j
    """
    out = np.zeros((8, 128, QG), np.float32)
    kv = np.arange(128)[:, None]
    q = np.arange(QG)[None, :]
    for s in range(8):
        # chunk kv offset relative to group q start g:
        # last 8 chunks start at span-1024 = g - (1024 - 512*j) ... for lane j:
        # offset_of_slot_s = (s - (8 - span_minus_g/128)) ... simpler:
        # lane j: g = span - 1024 + 512*j, chunk K0 = span - 1024 + 128*s
        # relative to g: K0 - g = 128*s - 512*j
        rel = 128 * s - 512 * j
        out[s] = np.where(rel + kv > q, np.float32(MASKVAL), np.float32(0.0))
    return out


def _run(inputs: dict, trace: bool = False, trace_kwargs: dict | None = None):
    x = np.asarray(inputs["x"], np.float32)
    Wk = np.asarray(inputs["Wk"], np.float32)
    Wq = np.asarray(inputs["Wq"], np.float32)
    Wv = np.asarray(inputs["Wv"], np.float32)

    nc = _get_program()

    wk16 = Wk.astype(NPBF16)
    wq16 = Wq.astype(NPBF16)
    wv16 = Wv.astype(NPBF16)
    msks = [_make_masks(j) for j in range(2)]

    in_maps = []
    for b in range(B):
        xtb = np.ascontiguousarray(x[b].T).astype(NPBF16)  # [C, T]
        for j in range(2):
            xtq = np.concatenate(
                [xtb[:, (2 * i + j) * QG:(2 * i + j + 1) * QG] for i in range(NG)],
                axis=1,
            )
            in_maps.append(
                {
                    "xt": xtb,
                    "xtq": np.ascontiguousarray(xtq),
                    "wk": wk16,
                    "wq": wq16,
                    "wv": wv16,
                    "msk": msks[j],
                }
            )

    res = run_bass_kernel_spmd(
        nc,
        in_maps,
        core_ids=list(range(NCORES)),
        trace=trace,
        **(trace_kwargs or {}),
    )

    out = np.empty((B, T, H), np.float32)
    for core in range(NCORES):
        b, j = divmod(core, 2)
        oT = np.asarray(res.results[core]["outT"], np.float32)  # [H, NG*QG]
        for i in range(NG):
            g = (2 * i + j) * QG
            out[b, g:g + QG, :] = oT[:, i * QG:(i + 1) * QG].T
    return out, res


def kernel(**inputs) -> np.ndarray:
    out, _ = _run(inputs, trace=False)
    return out
